# revision 3
# baseline (speedup 1.0000x reference)
"""GNN message-passing layer on 8 Trainium2 NeuronCores.

Strategy (node-sharded SPMD, no collectives):
  - Rewrite per-edge MLP: relu([nf[s], nf[t], ef] @ W1 + b1) @ W2
      = relu(A[s] + B[t] + C_e) @ W2  with A = nf@W1a, B = nf@W1b + b1,
        C = ef@W1c  (W1 split rows 0:128 / 128:256 / 256:320).
    W2 is linear, so it is applied AFTER mean-aggregation:
      agg = meanH @ W2 + b2*(cnt>0),  meanH = segsum(h * invcnt[t]).
    agg only feeds U1's second half, so fold M2 = W2 @ U1b and never
    materialize agg.
  - Each core owns 6272 consecutive nodes (28 windows x 224 nodes) and
    processes exactly the edges targeting them (host sorts edges by
    (core, window, src>=32768)).  Gathers of A rows use two dma_gather
    passes (int16 index limit); B rows are core-local.
  - Segment-sum via one-hot matmul: h_tile (128e x 128h, fp16) x
    S_tile (128e x 224w one-hot) accumulated in PSUM per window,
    copied into an SBUF aggT (128h x 6272n, f32r).
  - Node MLP in transposed layout with f32r wide matmuls; final
    residual + layernorm per 128-node tile; each core writes its own
    6272x128 output shard.
"""

import contextlib
import math
import os

import numpy as np

import concourse.bass as bass
import concourse.mybir as mybir
import concourse.tile as tile
from concourse import bacc
from concourse.bass import ds, ts
from concourse.bass_utils import run_bass_kernel_spmd

F32 = mybir.dt.float32
F32R = mybir.dt.float32r
F16 = mybir.dt.float16
I16 = mybir.dt.int16
I32 = mybir.dt.int32

N_NODES = 50000
N_EDGES = 800000
NODE_DIM = 128
EDGE_DIM = 64
HIDDEN = 128
EPS = 1e-5

N_CORES = 8
WIN = 224                 # nodes per aggregation window (PSUM free dim)
N_WIN = 28                # windows per core
NPC = WIN * N_WIN         # 6272 nodes per core
N_NODES_PAD = NPC * N_CORES   # 50176
SPLIT = 32768             # int16 index split for the A table
P = 128
D = 128                   # A/B row width

TRACE = False             # test.py sets True to collect a profile
LAST_RESULT = {}          # exec_time_ns etc. stashed here for test.py

_program_cache = {}


def _build_program(J_LO, J_HI):
    T = J_LO + J_HI
    N_LO, N_HI = J_LO * P, J_HI * P
    E_PAD = N_WIN * T * P
    NT_FULL = N_NODES_PAD // P      # 392 tiles for the A build
    NT_OWN = NPC // P               # 49
    ZC = 448                        # z1 chunk width (14 chunks)
    NZ = NPC // ZC

    nc = bacc.Bacc(
        "TRN2",
        target_bir_lowering=False,
        debug=False,
        enable_asserts=False,
        num_devices=N_CORES,
    )

    def din(name, shape, dtype):
        return nc.dram_tensor(name, shape, dtype, kind="ExternalInput").ap()

    nfT_full16 = din("nfT_full16", [P, N_NODES_PAD], F16)
    nfT_own = din("nfT_own", [P, NPC], F32R)
    nf_own = din("nf_own", [NPC, P], F32)
    idx_lo = din("idx_lo", [P, N_WIN * (N_LO // 16)], I16)
    idx_hi = din("idx_hi", [P, N_WIN * (N_HI // 16)], I16)
    idx_b = din("idx_b", [P, N_WIN * (T * P // 16)], I16)
    tgtl = din("tgtl", [P, N_WIN * T], F16)
    invc = din("invc", [P, N_WIN * T], F32)
    efT = din("efT", [EDGE_DIM, E_PAD], F16)
    cntpos = din("cntpos", [1, NPC], F32R)
    W1a16 = din("W1a16", [P, HIDDEN], F16)
    W1b = din("W1b", [P, HIDDEN], F32R)
    W1c16 = din("W1c16", [EDGE_DIM, HIDDEN], F16)
    b1row = din("b1row", [1, HIDDEN], F32R)
    W2T = din("W2T", [HIDDEN, HIDDEN], F32)
    U1b_t = din("U1b", [HIDDEN, HIDDEN], F32)
    U1a = din("U1a", [P, HIDDEN], F32R)
    b2col = din("b2col", [HIDDEN, 1], F32)
    c1col = din("c1col", [HIDDEN, 1], F32)
    c2row = din("c2row", [1, NODE_DIM], F32)
    gamma_in = din("gamma", [1, NODE_DIM], F32)
    beta_in = din("beta", [1, NODE_DIM], F32)
    U2_in = din("U2", [HIDDEN, NODE_DIM], F32)
    out_t = nc.dram_tensor("out", [NPC, NODE_DIM], F32, kind="ExternalOutput").ap()

    with tile.TileContext(nc) as tc, contextlib.ExitStack() as ctx:
        dram = ctx.enter_context(tc.tile_pool(name="dram", bufs=1, space="DRAM"))
        A_dram = dram.tile([N_NODES_PAD, D], F16)
        B_dram = dram.tile([NPC, D], F16)

        const = ctx.enter_context(tc.tile_pool(name="const", bufs=1))
        W1a16_sb = const.tile([P, HIDDEN], F16)
        nc.sync.dma_start(out=W1a16_sb, in_=W1a16)
        W1b_sb = const.tile([P, HIDDEN], F32R)
        nc.sync.dma_start(out=W1b_sb, in_=W1b)
        W1c16_sb = const.tile([EDGE_DIM, HIDDEN], F16)
        nc.sync.dma_start(out=W1c16_sb, in_=W1c16)
        b1_sb = const.tile([1, HIDDEN], F32R)
        nc.sync.dma_start(out=b1_sb, in_=b1row)
        ones1_f32 = const.tile([1, P], F32)
        nc.vector.memset(ones1_f32, 1.0)
        ones1_sb = const.tile([1, P], F32R)
        nc.vector.tensor_copy(out=ones1_sb[:], in_=ones1_f32[:])
        nfT_own_sb = const.tile([P, NPC], F32R)
        nc.sync.dma_start(out=nfT_own_sb, in_=nfT_own)

        iota_i = const.tile([P, WIN], I32)
        nc.gpsimd.iota(iota_i[:], pattern=[[1, WIN]], base=0, channel_multiplier=0)
        iota16 = const.tile([P, WIN], F16)
        nc.vector.tensor_copy(out=iota16[:], in_=iota_i[:])

        # ---------------- Phase 0: build A and B tables in DRAM ----------
        with tc.tile_pool(name="p0", bufs=4) as p0, \
             tc.tile_pool(name="psum0", bufs=4, space="PSUM") as psum0:
            for i in range(NT_FULL):
                nft = p0.tile([P, P], F16, tag="nft")
                nc.sync.dma_start(out=nft, in_=nfT_full16[:, ts(i, P)])
                aps = psum0.tile([P, D], F32, tag="aps")
                nc.tensor.matmul(out=aps[:], lhsT=nft[:], rhs=W1a16_sb[:],
                                 start=True, stop=True)
                asb = p0.tile([P, D], F16, tag="asb")
                nc.scalar.copy(out=asb[:], in_=aps[:])
                nc.sync.dma_start(out=A_dram[ts(i, P), :], in_=asb[:])
            for i in range(NT_OWN):
                bps = psum0.tile([P, D], F32, tag="aps")
                nc.tensor.matmul(out=bps[:], lhsT=nfT_own_sb[:, ts(i, P)],
                                 rhs=W1b_sb[:], start=True, stop=False)
                nc.tensor.matmul(out=bps[:], lhsT=ones1_sb[:], rhs=b1_sb[:],
                                 start=False, stop=True)
                bsb = p0.tile([P, D], F16, tag="asb")
                nc.scalar.copy(out=bsb[:], in_=bps[:])
                nc.sync.dma_start(out=B_dram[ts(i, P), :], in_=bsb[:])

        # resident edge-phase inputs
        idx_lo_sb = const.tile([P, N_WIN * (N_LO // 16)], I16)
        nc.sync.dma_start(out=idx_lo_sb, in_=idx_lo)
        idx_hi_sb = const.tile([P, N_WIN * (N_HI // 16)], I16)
        nc.sync.dma_start(out=idx_hi_sb, in_=idx_hi)
        idx_b_sb = const.tile([P, N_WIN * (T * P // 16)], I16)
        nc.sync.dma_start(out=idx_b_sb, in_=idx_b)
        tgtl_sb = const.tile([P, N_WIN * T], F16)
        nc.sync.dma_start(out=tgtl_sb, in_=tgtl)
        invc_sb = const.tile([P, N_WIN * T], F32)
        nc.sync.dma_start(out=invc_sb, in_=invc)

        aggT = const.tile([P, NPC], F32R)

        # ---------------- Phase 1: edge phase ----------------------------
        with tc.tile_pool(name="pq", bufs=2) as pq, \
             tc.tile_pool(name="pef", bufs=2) as pef, \
             tc.tile_pool(name="ph", bufs=4) as ph, \
             tc.tile_pool(name="pps", bufs=4, space="PSUM") as pps, \
             tc.tile_pool(name="pwin", bufs=2, space="PSUM") as pwin:
            for w in range(N_WIN):
                qa = pq.tile([P, T * D], F16, tag="qa")
                qb = pq.tile([P, T * D], F16, tag="qb")
                efs = pef.tile([EDGE_DIM, T * P], F16, tag="efs")
                nc.sync.dma_start(out=efs, in_=efT[:, w * T * P:(w + 1) * T * P])
                nc.gpsimd.dma_gather(
                    qa[:, :N_LO].rearrange("p (j d) -> p j d", d=D),
                    A_dram[:SPLIT, :],
                    idx_lo_sb[:, w * (N_LO // 16):(w + 1) * (N_LO // 16)],
                    N_LO, N_LO, D, single_packet=False)
                nc.gpsimd.dma_gather(
                    qa[:, N_LO:].rearrange("p (j d) -> p j d", d=D),
                    A_dram[SPLIT:, :],
                    idx_hi_sb[:, w * (N_HI // 16):(w + 1) * (N_HI // 16)],
                    N_HI, N_HI, D, single_packet=False)
                nc.gpsimd.dma_gather(
                    qb[:, :].rearrange("p (j d) -> p j d", d=D),
                    B_dram[:, :],
                    idx_b_sb[:, w * (T * P // 16):(w + 1) * (T * P // 16)],
                    T * P, T * P, D, single_packet=False)
                wps = pwin.tile([P, WIN], F32, tag="wps")
                for j in range(T):
                    col = w * T + j
                    cps = pps.tile([P, HIDDEN], F32, tag="cps")
                    nc.tensor.matmul(out=cps[:], lhsT=efs[:, ts(j, P)],
                                     rhs=W1c16_sb[:], start=True, stop=True)
                    q2 = ph.tile([P, HIDDEN], F16, tag="q2")
                    nc.vector.tensor_add(out=q2[:], in0=qa[:, ts(j, D)],
                                         in1=qb[:, ts(j, D)])
                    q3 = ph.tile([P, HIDDEN], F16, tag="q3")
                    nc.vector.tensor_add(out=q3[:], in0=q2[:], in1=cps[:])
                    h = ph.tile([P, HIDDEN], F16, tag="h")
                    nc.scalar.activation(
                        out=h[:], in_=q3[:],
                        func=mybir.ActivationFunctionType.Relu,
                        scale=invc_sb[:, col:col + 1])
                    S = ph.tile([P, WIN], F16, tag="S")
                    nc.vector.tensor_tensor(
                        out=S[:],
                        in0=tgtl_sb[:, col:col + 1].to_broadcast([P, WIN]),
                        in1=iota16[:],
                        op=mybir.AluOpType.is_equal)
                    nc.tensor.matmul(out=wps[:], lhsT=h[:], rhs=S[:],
                                     start=(j == 0), stop=(j == T - 1))
                nc.vector.tensor_copy(out=aggT[:, w * WIN:(w + 1) * WIN],
                                      in_=wps[:])

        # ---------------- Phase 2: node MLP + layernorm -------------------
        with tc.tile_pool(name="pn", bufs=4) as pn, \
             tc.tile_pool(name="pnc", bufs=1) as pnc, \
             tc.tile_pool(name="psz", bufs=2, space="PSUM") as psz:
            W2T_sb = pnc.tile([HIDDEN, HIDDEN], F32)
            nc.sync.dma_start(out=W2T_sb, in_=W2T)
            U1b_sb = pnc.tile([HIDDEN, HIDDEN], F32)
            nc.sync.dma_start(out=U1b_sb, in_=U1b_t)
            U1a_sb = pnc.tile([P, HIDDEN], F32R)
            nc.sync.dma_start(out=U1a_sb, in_=U1a)
            b2col_sb = pnc.tile([HIDDEN, 1], F32)
            nc.sync.dma_start(out=b2col_sb, in_=b2col)
            c1col_sb = pnc.tile([HIDDEN, 1], F32)
            nc.sync.dma_start(out=c1col_sb, in_=c1col)
            U2_sb = pnc.tile([HIDDEN, NODE_DIM], F32)
            nc.sync.dma_start(out=U2_sb, in_=U2_in)
            cntpos_sb = pnc.tile([1, NPC], F32R)
            nc.sync.dma_start(out=cntpos_sb, in_=cntpos)

            m2ps = psz.tile([HIDDEN, HIDDEN], F32, tag="zps")
            nc.tensor.matmul(out=m2ps[:], lhsT=W2T_sb[:], rhs=U1b_sb[:],
                             start=True, stop=True)
            M2_sb = pnc.tile([HIDDEN, HIDDEN], F32R)
            nc.vector.tensor_copy(out=M2_sb[:], in_=m2ps[:])
            b2ups = psz.tile([1, HIDDEN], F32, tag="zps")
            nc.tensor.matmul(out=b2ups[:], lhsT=b2col_sb[:], rhs=U1b_sb[:],
                             start=True, stop=True)
            b2u_sb = pnc.tile([1, HIDDEN], F32R)
            nc.vector.tensor_copy(out=b2u_sb[:], in_=b2ups[:])

            z1T = pnc.tile([HIDDEN, NPC], F32)
            for c in range(NZ):
                sl = ts(c, ZC)
                zps = psz.tile([HIDDEN, ZC], F32, tag="zps")
                nc.tensor.matmul(out=zps[:], lhsT=M2_sb[:], rhs=aggT[:, sl],
                                 start=True, stop=False)
                nc.tensor.matmul(out=zps[:], lhsT=U1a_sb[:],
                                 rhs=nfT_own_sb[:, sl], start=False, stop=False)
                nc.tensor.matmul(out=zps[:], lhsT=b2u_sb[:],
                                 rhs=cntpos_sb[:, sl], start=False, stop=True)
                nc.scalar.activation(out=z1T[:, sl], in_=zps[:],
                                     func=mybir.ActivationFunctionType.Relu,
                                     bias=c1col_sb[:, :])

            c2g = pnc.tile([P, NODE_DIM], F32)
            nc.gpsimd.dma_start(out=c2g, in_=c2row.to_broadcast([P, NODE_DIM]))
            gammag = pnc.tile([P, NODE_DIM], F32)
            nc.gpsimd.dma_start(out=gammag, in_=gamma_in.to_broadcast([P, NODE_DIM]))
            betag = pnc.tile([P, NODE_DIM], F32)
            nc.gpsimd.dma_start(out=betag, in_=beta_in.to_broadcast([P, NODE_DIM]))
            eps_sb = pnc.tile([P, 1], F32)
            nc.vector.memset(eps_sb, EPS)

            for i in range(NT_OWN):
                ups = psz.tile([P, NODE_DIM], F32, tag="ups")
                nc.tensor.matmul(out=ups[:], lhsT=z1T[:, ts(i, P)], rhs=U2_sb[:],
                                 start=True, stop=True)
                nft2 = pn.tile([P, NODE_DIM], F32, tag="nft2")
                nc.sync.dma_start(out=nft2, in_=nf_own[ts(i, P), :])
                x = pn.tile([P, NODE_DIM], F32, tag="x")
                nc.vector.tensor_add(out=x[:], in0=ups[:], in1=nft2[:])
                nc.vector.tensor_add(out=x[:], in0=x[:], in1=c2g[:])
                stats = pn.tile([P, 6], F32, tag="stats")
                nc.vector.bn_stats(out=stats[:], in_=x[:])
                mv = pn.tile([P, 2], F32, tag="mv")
                nc.vector.bn_aggr(out=mv[:], in_=stats[:])
                stdv = pn.tile([P, 1], F32, tag="stdv")
                nc.scalar.activation(out=stdv[:], in_=mv[:, 1:2],
                                     func=mybir.ActivationFunctionType.Sqrt,
                                     bias=eps_sb[:, :])
                rstd = pn.tile([P, 1], F32, tag="rstd")
                nc.vector.reciprocal(out=rstd[:], in_=stdv[:])
                nc.vector.tensor_scalar(
                    out=x[:], in0=x[:], scalar1=mv[:, 0:1], scalar2=rstd[:, :],
                    op0=mybir.AluOpType.subtract, op1=mybir.AluOpType.mult)
                nc.vector.tensor_tensor(out=x[:], in0=x[:], in1=gammag[:],
                                        op=mybir.AluOpType.mult)
                o = pn.tile([P, NODE_DIM], F32, tag="o")
                nc.vector.tensor_tensor(out=o[:], in0=x[:], in1=betag[:],
                                        op=mybir.AluOpType.add)
                nc.sync.dma_start(out=out_t[ts(i, P), :], in_=o[:])

    nc.compile()
    return nc


def _preprocess(inputs):
    nf = np.ascontiguousarray(np.asarray(inputs["node_features"], np.float32))
    ei = np.asarray(inputs["edge_index"])
    ef = np.ascontiguousarray(np.asarray(inputs["edge_features"], np.float32))
    W1 = np.asarray(inputs["W1"], np.float32)
    b1 = np.asarray(inputs["b1"], np.float32)
    W2 = np.asarray(inputs["W2"], np.float32)
    b2 = np.asarray(inputs["b2"], np.float32)
    U1 = np.asarray(inputs["U1"], np.float32)
    c1 = np.asarray(inputs["c1"], np.float32)
    U2 = np.asarray(inputs["U2"], np.float32)
    c2 = np.asarray(inputs["c2"], np.float32)
    gamma = np.asarray(inputs["gamma"], np.float32)
    beta = np.asarray(inputs["beta"], np.float32)

    E = ei.shape[1]
    src = ei[0].astype(np.int64)
    tgt = ei[1].astype(np.int64)
    core = tgt // NPC
    tic = tgt - core * NPC              # target index within core
    win = tic // WIN
    hi = (src >= SPLIT).astype(np.int64)

    order = np.lexsort((hi, win, core))
    src_s = src[order]
    hi_s = hi[order]
    core_s = core[order]
    win_s = win[order]
    tic_s = tic[order]

    gid = (core_s * N_WIN + win_s) * 2 + hi_s
    counts = np.bincount(gid, minlength=N_CORES * N_WIN * 2)
    lo_max = int(counts[0::2].max())
    hi_max = int(counts[1::2].max())
    J_LO = max(1, math.ceil(lo_max / P))
    J_HI = max(1, math.ceil(hi_max / P))
    T = J_LO + J_HI
    N_LO, N_HI = J_LO * P, J_HI * P
    E_PAD = N_WIN * T * P

    gstart = np.zeros(counts.shape[0], np.int64)
    np.cumsum(counts[:-1], out=gstart[1:])
    rank = np.arange(E, dtype=np.int64) - gstart[gid]
    slot_in_win = np.where(hi_s == 1, N_LO, 0) + rank
    slot = win_s * (T * P) + slot_in_win

    # per-(core,window) gather index lists
    lo_idx = np.zeros((N_CORES, N_WIN, N_LO), np.int16)
    hi_idx = np.zeros((N_CORES, N_WIN, N_HI), np.int16)
    lom = hi_s == 0
    lo_idx[core_s[lom], win_s[lom], rank[lom]] = src_s[lom].astype(np.int16)
    him = ~lom
    hi_idx[core_s[him], win_s[him], rank[him]] = (src_s[him] - SPLIT).astype(np.int16)
    b_idx = np.zeros((N_CORES, N_WIN, T * P), np.int16)
    b_idx[core_s, win_s, slot_in_win] = tic_s.astype(np.int16)

    tgtl_a = np.full((N_CORES, N_WIN, T * P), -1.0, np.float16)
    tgtl_a[core_s, win_s, slot_in_win] = (tic_s - win_s * WIN).astype(np.float16)

    cnt = np.bincount(tgt, minlength=N_NODES_PAD)
    invc_node = (1.0 / np.maximum(cnt, 1)).astype(np.float32)
    invc_a = np.ones((N_CORES, N_WIN, T * P), np.float32)
    invc_a[core_s, win_s, slot_in_win] = invc_node[tgt[order]]

    eft = np.zeros((N_CORES, E_PAD, EDGE_DIM), np.float16)
    eft[core_s, slot] = ef[order].astype(np.float16)
    eft = np.ascontiguousarray(eft.transpose(0, 2, 1))     # (C, 64, E_PAD)

    def wrap16(arr):
        C_, NW_, L = arr.shape
        a = arr.reshape(C_, NW_, L // 16, 16).transpose(0, 3, 1, 2)
        a = np.ascontiguousarray(a.reshape(C_, 16, NW_ * (L // 16)))
        return np.ascontiguousarray(np.tile(a, (1, 8, 1)))

    idx_lo_in = wrap16(lo_idx)
    idx_hi_in = wrap16(hi_idx)
    idx_b_in = wrap16(b_idx)
    tgtl_in = np.ascontiguousarray(
        tgtl_a.reshape(N_CORES, N_WIN * T, P).transpose(0, 2, 1))
    invc_in = np.ascontiguousarray(
        invc_a.reshape(N_CORES, N_WIN * T, P).transpose(0, 2, 1))

    nfp = np.zeros((N_NODES_PAD, NODE_DIM), np.float32)
    nfp[:N_NODES] = nf
    nfT_full16 = np.ascontiguousarray(nfp.T).astype(np.float16)
    cntpos_all = (cnt > 0).astype(np.float32)

    shared = {
        "nfT_full16": nfT_full16,
        "W1a16": W1[0:128].astype(np.float16),
        "W1b": np.ascontiguousarray(W1[128:256]),
        "W1c16": W1[256:320].astype(np.float16),
        "b1row": b1[None, :].copy(),
        "W2T": np.ascontiguousarray(W2.T),
        "U1b": np.ascontiguousarray(U1[128:256]),
        "U1a": np.ascontiguousarray(U1[0:128]),
        "b2col": b2[:, None].copy(),
        "c1col": c1[:, None].copy(),
        "c2row": c2[None, :].copy(),
        "gamma": gamma[None, :].copy(),
        "beta": beta[None, :].copy(),
        "U2": U2.copy(),
    }
    in_maps = []
    for c in range(N_CORES):
        sl = slice(c * NPC, (c + 1) * NPC)
        m = dict(shared)
        m.update({
            "nfT_own": np.ascontiguousarray(nfp[sl].T),
            "nf_own": nfp[sl].copy(),
            "idx_lo": idx_lo_in[c],
            "idx_hi": idx_hi_in[c],
            "idx_b": idx_b_in[c],
            "tgtl": tgtl_in[c],
            "invc": invc_in[c],
            "efT": eft[c],
            "cntpos": np.ascontiguousarray(cntpos_all[None, sl]),
        })
        in_maps.append(m)
    return (J_LO, J_HI), in_maps


def kernel(**inputs) -> np.ndarray:
    cfg, in_maps = _preprocess(inputs)
    if cfg not in _program_cache:
        _program_cache[cfg] = _build_program(*cfg)
    nc = _program_cache[cfg]
    res = run_bass_kernel_spmd(
        nc, in_maps, core_ids=list(range(N_CORES)), trace=TRACE)
    LAST_RESULT["exec_time_ns"] = res.exec_time_ns
    LAST_RESULT["trace"] = res.instructions_and_trace
    out = np.concatenate([r["out"] for r in res.results], axis=0)
    return np.ascontiguousarray(out[:N_NODES]).astype(np.float32)
